# revision 1
# baseline (speedup 1.0000x reference)
"""Trainium2 Bass kernel for nn_AttentionBlock (pre-LN MHA with ALiBi +
pre-LN SwiGLU FFN), distributed over 8 NeuronCores.

Sharding: core = (batch, head-half). Each core computes LN1 + QKV +
attention for 8 of 16 heads over its batch's full 2048 rows, a partial
O-projection (its 512 of 1024 contraction dims), then pairwise
on-device ReduceScatters combine the partials and each core finishes
LN2 + SwiGLU FFN for half of its batch's rows.

ALiBi + causal handling: the additive alibi term s_hat*(c-r) is folded
into the score matmul via 4 augmentation rows (a c_lo/c_hi/r_lo/r_hi
split keeps every bf16 product exact); scores are computed transposed
[k, q], exponentiated without max subtraction (scores are bounded), the
causal diagonal is masked post-exp with affine_select, and the softmax
denominator comes from a ones-column appended to V.

kernel(**inputs) takes the full unsharded inputs of
reference.setup_inputs() and returns the full (4, 2048, 1024) output.
"""

import os
from contextlib import ExitStack

import numpy as np
import ml_dtypes

import concourse.bass as bass
import concourse.bacc as bacc
import concourse.mybir as mybir
import concourse.tile as tile
from concourse.masks import make_identity
from concourse.bass_utils import run_bass_kernel_spmd

BFNP = ml_dtypes.bfloat16
bf = lambda a: np.asarray(a).astype(BFNP)
f32 = lambda a: np.asarray(a, np.float32)

FP32 = mybir.dt.float32
BF16 = mybir.dt.bfloat16
AF = mybir.ActivationFunctionType

S = 2048
N_CORES = 8          # rows per batch
D = 1024          # model dim
NH = 8            # heads per core
DH = 64           # head dim
DV = NH * DH      # 512, per-core qkv dim
ROWS_OUT = 1024   # rows per core after ReduceScatter
NKT = S // 128    # 16 k-tiles
PAIRS = [[0, 1], [2, 3], [4, 5], [6, 7]]


def _build_kernel(nc):
    x = nc.dram_tensor("x", [S, D], FP32, kind="ExternalInput").ap()
    x_my = nc.dram_tensor("x_my", [ROWS_OUT, D], FP32, kind="ExternalInput").ap()
    wqT = nc.dram_tensor("wqT", [D, DV], BF16, kind="ExternalInput").ap()
    wkT = nc.dram_tensor("wkT", [D, DV], BF16, kind="ExternalInput").ap()
    wvT = nc.dram_tensor("wvT", [D, DV], BF16, kind="ExternalInput").ap()
    woT = nc.dram_tensor("woT", [DV, D], BF16, kind="ExternalInput").ap()
    w1T = nc.dram_tensor("w1T", [D, D], BF16, kind="ExternalInput").ap()
    w3T = nc.dram_tensor("w3T", [D, D], BF16, kind="ExternalInput").ap()
    w2T = nc.dram_tensor("w2T", [D, D], BF16, kind="ExternalInput").ap()
    qkvbias = nc.dram_tensor("qkvbias", [3, DV], FP32, kind="ExternalInput").ap()
    b13 = nc.dram_tensor("b13", [2, D], FP32, kind="ExternalInput").ap()
    qext = nc.dram_tensor("qext", [NH, 4, S], BF16, kind="ExternalInput").ap()
    kext = nc.dram_tensor("kext", [NH, 4, S], BF16, kind="ExternalInput").ap()

    y = nc.dram_tensor("y", [ROWS_OUT, D], FP32, kind="ExternalOutput").ap()
    with tile.TileContext(nc) as tc, ExitStack() as ctx:
        singles = ctx.enter_context(tc.tile_pool(name="singles", bufs=1))
        dram = ctx.enter_context(tc.tile_pool(name="dram", bufs=1, space="DRAM"))

        ident = singles.tile([128, 128], BF16)
        make_identity(nc, ident)

        eps = singles.tile([128, 1], FP32)
        nc.vector.memset(eps, 1e-5)

        # per-partition biases for Q/K evac ([128, 4] : dout-in-chunk x chunk)
        qb_sb = singles.tile([128, 4], FP32)
        kb_sb = singles.tile([128, 4], FP32)
        nc.sync.dma_start(out=qb_sb, in_=qkvbias[0].rearrange("(c p) -> p c", p=128))
        nc.sync.dma_start(out=kb_sb, in_=qkvbias[1].rearrange("(c p) -> p c", p=128))
        # V bias broadcast along partitions [128, 512]
        vb_sb = singles.tile([128, DV], FP32)
        nc.sync.dma_start(
            out=vb_sb,
            in_=bass.AP(tensor=qkvbias.tensor, offset=2 * DV, ap=[[0, 128], [1, DV]]),
        )
        b1_sb = singles.tile([128, 8], FP32)
        b3_sb = singles.tile([128, 8], FP32)
        nc.sync.dma_start(out=b1_sb, in_=b13[0].rearrange("(c p) -> p c", p=128))
        nc.sync.dma_start(out=b3_sb, in_=b13[1].rearrange("(c p) -> p c", p=128))

        # big tensors for phases 1-4 (one pool, closed after the O-proj)
        p_big_cm = tc.tile_pool(name="p_big", bufs=1)
        p_big = p_big_cm.__enter__()

        hTg = [p_big.tile([128, 8, 512], BF16, name=f"hT{g}", tag=f"hT{g}")
               for g in range(4)]                    # h^T row-groups of 512
        qaug = [p_big.tile([68, S], BF16, name=f"qaug{h}", tag=f"qaug{h}") for h in range(NH)]
        kaug = [p_big.tile([68, S], BF16, name=f"kaug{h}", tag=f"kaug{h}") for h in range(NH)]
        vsb = p_big.tile([128, NKT, NH * 65], BF16)  # v rows + ones col per head
        oT = p_big.tile([128, 4, S], BF16)
        o_rm = p_big.tile([128, NKT, DV], BF16)      # o row-major staging

        # ---------------- Phases 1+2 fused: LN1 + h^T + QKV, row-group interleaved ----------------
        with tc.tile_pool(name="ln1", bufs=4) as ln1p, \
             tc.tile_pool(name="xgp", bufs=3) as xgp, \
             tc.tile_pool(name="ptr1", bufs=4, space="PSUM") as ptr1, \
             tc.tile_pool(name="qkvw", bufs=1) as qkvw, \
             tc.tile_pool(name="pmm2", bufs=3, space="PSUM") as pmm2:
            wq_sb = qkvw.tile([128, 8, DV], BF16, tag="wq")
            wk_sb = qkvw.tile([128, 8, DV], BF16, tag="wk")
            wv_sb = qkvw.tile([128, 8, DV], BF16, tag="wv")
            nc.gpsimd.memset(vsb, 1.0)

            for g in range(4):
                xgs = []
                for gh in range(2):
                    xg = xgp.tile([128, 2, D], FP32, tag="xg",
                                  name=f"xg{g}_{gh}")
                    nc.sync.dma_start(
                        out=xg,
                        in_=x[g * 512 + gh * 256:g * 512 + gh * 256 + 256, :]
                            .rearrange("(a p) d -> p a d", p=128))
                    xgs.append(xg)
                for rt in range(4 * g, 4 * g + 4):
                    xt = xgs[(rt % 4) // 2][:, rt % 2, :]
                    stats = ln1p.tile([128, 2, 6], FP32, tag="stats")
                    xr = xt.rearrange("p (s f) -> p s f", s=2)
                    nc.vector.bn_stats(out=stats[:, 0, :], in_=xr[:, 0, :])
                    nc.vector.bn_stats(out=stats[:, 1, :], in_=xr[:, 1, :])
                    mv = ln1p.tile([128, 2], FP32, tag="mv")
                    nc.vector.bn_aggr(out=mv, in_=stats)
                    rstd = ln1p.tile([128, 1], FP32, tag="rstd")
                    nc.scalar.activation(out=rstd, in_=mv[:, 1:2], func=AF.Sqrt,
                                         bias=eps)
                    nc.vector.reciprocal(out=rstd, in_=rstd)
                    hrow = ln1p.tile([128, D], BF16, tag="hrow")
                    nc.vector.tensor_scalar(
                        out=hrow, in0=xt, scalar1=mv[:, 0:1], scalar2=rstd,
                        op0=mybir.AluOpType.subtract, op1=mybir.AluOpType.mult,
                    )
                    for c in range(8):
                        pt = ptr1.tile([128, 128], BF16, tag="pt")
                        nc.tensor.transpose(pt, hrow[:, c * 128:(c + 1) * 128], ident)
                        nc.scalar.copy(
                            out=hTg[rt // 4][:, c, (rt % 4) * 128:(rt % 4) * 128 + 128],
                            in_=pt)

                if g == 0:
                    nc.gpsimd.dma_start(out=wq_sb, in_=wqT.rearrange("(c p) n -> p c n", p=128))
                    nc.gpsimd.dma_start(out=wk_sb, in_=wkT.rearrange("(c p) n -> p c n", p=128))
                    nc.gpsimd.dma_start(out=wv_sb, in_=wvT.rearrange("(c p) n -> p c n", p=128))
                    for h in range(NH):
                        nc.gpsimd.dma_start(out=qaug[h][64:68, :], in_=qext[h])
                        nc.gpsimd.dma_start(out=kaug[h][64:68, :], in_=kext[h])
                # Q/K for this row group
                for (w_sb, aug, bias, scale) in (
                    (wq_sb, qaug, qb_sb, 0.125),
                    (wk_sb, kaug, kb_sb, 1.0),
                ):
                    for m in range(4):
                        ps = pmm2.tile([128, 512], FP32, tag="ps")
                        for c in range(8):
                            nc.tensor.matmul(
                                ps,
                                lhsT=w_sb[:, c, m * 128:(m + 1) * 128],
                                rhs=hTg[g][:, c, :],
                                start=(c == 0), stop=(c == 7),
                            )
                        for sub in range(2):
                            nc.scalar.activation(
                                out=aug[2 * m + sub][0:64, g * 512:(g + 1) * 512],
                                in_=ps[sub * 64:(sub + 1) * 64, :],
                                func=AF.Identity,
                                bias=bias[sub * 64:(sub + 1) * 64, m:m + 1],
                                scale=scale,
                            )
                # V for this row group (row-major out)
                for kt in range(4 * g, 4 * g + 4):
                    ps = pmm2.tile([128, 512], FP32, tag="ps")
                    for c in range(8):
                        nc.tensor.matmul(
                            ps,
                            lhsT=hTg[kt // 4][:, c, (kt % 4) * 128:(kt % 4) * 128 + 128],
                            rhs=wv_sb[:, c, :],
                            start=(c == 0), stop=(c == 7),
                        )
                    out_ap = vsb[:, kt, :].rearrange("p (h e) -> p h e", h=NH)[:, :, 0:64]
                    in_ap = ps.rearrange("p (h e) -> p h e", h=NH)
                    nc.vector.scalar_tensor_tensor(
                        out=out_ap, in0=in_ap, scalar=1.0,
                        in1=vb_sb.rearrange("p (h e) -> p h e", h=NH),
                        op0=mybir.AluOpType.mult, op1=mybir.AluOpType.add,
                    )

        # ---------------- Phase 3: attention (head pairs interleaved) ----------------
        with tc.tile_pool(name="att", bufs=4) as attp, \
             tc.tile_pool(name="psc", bufs=2, space="PSUM") as psc, \
             tc.tile_pool(name="poa", bufs=4, space="PSUM") as poa:
            for hp in range(NH // 2):
                heads = (2 * hp, 2 * hp + 1)
                for qt in range(8):          # 256-wide q tiles
                    nkt = 2 * (qt + 1)
                    oacc = [poa.tile([128, 65], FP32, name=f"oa{hp}_{qt}_{ii}",
                                     tag="oacc") for ii in range(4)]
                    for g0 in range(0, nkt, 4):
                        w = min(4, nkt - g0)
                        ats = []
                        for idx, h in enumerate(heads):
                            sc = psc.tile([128, 1024], FP32, tag="sc",
                                          name=f"sc{hp}_{qt}_{g0}_{idx}")
                            for i in range(w):
                                kt = g0 + i
                                nc.tensor.matmul(
                                    sc[:, i * 256:(i + 1) * 256],
                                    lhsT=kaug[h][:, kt * 128:(kt + 1) * 128],
                                    rhs=qaug[h][:, qt * 256:(qt + 1) * 256],
                                    start=True, stop=True,
                                )
                            at = attp.tile([128, 1024], BF16, tag="at",
                                           name=f"at{hp}_{qt}_{g0}_{idx}")
                            nc.scalar.activation(
                                out=at[:, :w * 256], in_=sc[:, :w * 256], func=AF.Exp,
                            )
                            for i in range(w):
                                kt = g0 + i
                                if kt >= nkt - 2:  # diagonal tiles: mask c > r
                                    nc.gpsimd.affine_select(
                                        out=at[:, i * 256:(i + 1) * 256],
                                        in_=at[:, i * 256:(i + 1) * 256],
                                        compare_op=mybir.AluOpType.is_ge,
                                        fill=0.0,
                                        base=qt * 256 - kt * 128,
                                        channel_multiplier=-1,
                                        pattern=[[1, 256]],
                                    )
                            ats.append(at)
                        for idx, h in enumerate(heads):
                            at = ats[idx]
                            for sub in range(2):
                                for i in range(w):
                                    kt = g0 + i
                                    nc.tensor.matmul(
                                        oacc[2 * idx + sub],
                                        lhsT=at[:, i * 256 + sub * 128:
                                                i * 256 + sub * 128 + 128],
                                        rhs=vsb[:, kt, h * 65:(h + 1) * 65],
                                        start=(kt == 0), stop=(kt == nkt - 1),
                                    )
                    for idx, h in enumerate(heads):
                        for sub in range(2):
                            oa = oacc[2 * idx + sub]
                            rec = attp.tile([128, 1], FP32, tag="rec")
                            nc.vector.reciprocal(out=rec, in_=oa[:, 64:65])
                            rr = qt * 2 + sub
                            nc.vector.tensor_scalar_mul(
                                out=o_rm[:, rr, h * 64:(h + 1) * 64],
                                in0=oa[:, 0:64], scalar1=rec,
                            )

        # Phase 3.5: transpose o to feature-major
        with tc.tile_pool(name="ptr35", bufs=4, space="PSUM") as ptr35:
            for rr in range(NKT):
                for c in range(4):
                    pt = ptr35.tile([128, 128], BF16, tag="pt")
                    nc.tensor.transpose(pt, o_rm[:, rr, c * 128:(c + 1) * 128], ident)
                    nc.scalar.copy(out=oT[:, c, rr * 128:(rr + 1) * 128], in_=pt)

        # ---------------- Phase 4: O-proj + ReduceScatter ----------------
        ccin = dram.tile([S, D], BF16)
        ccout = dram.tile([ROWS_OUT, D], BF16)
        with tc.tile_pool(name="wop", bufs=1) as wop, \
             tc.tile_pool(name="oproj", bufs=3) as op, \
             tc.tile_pool(name="pmm4", bufs=3, space="PSUM") as pmm4:
            wo_sb = wop.tile([128, 4, D], BF16, tag="wo")
            nc.sync.dma_start(out=wo_sb, in_=woT.rearrange("(c p) n -> p c n", p=128))
            for rt in range(S // 128):
                row_sb = op.tile([128, D], BF16, tag="row")
                for n in range(2):
                    ps = pmm4.tile([128, 512], FP32, tag="ps")
                    for c in range(4):
                        nc.tensor.matmul(
                            ps,
                            lhsT=oT[:, c, rt * 128:(rt + 1) * 128],
                            rhs=wo_sb[:, c, n * 512:(n + 1) * 512],
                            start=(c == 0), stop=(c == 3),
                        )
                    nc.scalar.copy(out=row_sb[:, n * 512:(n + 1) * 512], in_=ps)
                nc.sync.dma_start(out=ccin[rt * 128:(rt + 1) * 128, :], in_=row_sb)
            nc.gpsimd.collective_compute(
                "ReduceScatter",
                mybir.AluOpType.add,
                ins=[ccin.opt()],
                outs=[ccout.opt()],
                replica_groups=PAIRS,
            )

        p_big_cm.__exit__(None, None, None)
        p_x2_cm = tc.tile_pool(name="p_x2", bufs=1)          # phases 5-6
        p_x2 = p_x2_cm.__enter__()
        x2_sb = p_x2.tile([128, 8, D], FP32)
        h2g = [p_x2.tile([128, 8, 512], BF16, name=f"h2T{g}", tag=f"h2T{g}")
               for g in range(2)]


        # ---------------- Phase 5: x2 + LN2 + h2^T ----------------
        with tc.tile_pool(name="ln2", bufs=3) as ln2p, \
             tc.tile_pool(name="ptr5", bufs=4, space="PSUM") as ptr5:
            for rt in range(ROWS_OUT // 128):
                xt = ln2p.tile([128, D], FP32, tag="xt")
                nc.sync.dma_start(out=xt, in_=x_my[rt * 128:(rt + 1) * 128, :])
                rs = ln2p.tile([128, D], BF16, tag="rs")
                nc.sync.dma_start(out=rs, in_=ccout[rt * 128:(rt + 1) * 128, :])
                nc.vector.tensor_add(x2_sb[:, rt, :], xt, rs)
                stats = ln2p.tile([128, 2, 6], FP32, tag="stats")
                x2r = x2_sb[:, rt, :].rearrange("p (s f) -> p s f", s=2)
                nc.vector.bn_stats(out=stats[:, 0, :], in_=x2r[:, 0, :])
                nc.vector.bn_stats(out=stats[:, 1, :], in_=x2r[:, 1, :])
                mv = ln2p.tile([128, 2], FP32, tag="mv")
                nc.vector.bn_aggr(out=mv, in_=stats)
                rstd = ln2p.tile([128, 1], FP32, tag="rstd")
                nc.scalar.activation(out=rstd, in_=mv[:, 1:2], func=AF.Sqrt, bias=eps)
                nc.vector.reciprocal(out=rstd, in_=rstd)
                hrow = ln2p.tile([128, D], BF16, tag="hrow")
                nc.vector.tensor_scalar(
                    out=hrow, in0=x2_sb[:, rt, :], scalar1=mv[:, 0:1], scalar2=rstd,
                    op0=mybir.AluOpType.subtract, op1=mybir.AluOpType.mult,
                )
                for c in range(8):
                    pt = ptr5.tile([128, 128], BF16, tag="pt")
                    nc.tensor.transpose(pt, hrow[:, c * 128:(c + 1) * 128], ident)
                    nc.scalar.copy(
                        out=h2g[rt // 4][:, c, (rt % 4) * 128:(rt % 4) * 128 + 128],
                        in_=pt)

        # ---------------- Phase 6: FFN ----------------
        with tc.tile_pool(name="ffnw", bufs=1) as ffnw, \
             tc.tile_pool(name="ffn2", bufs=3) as ffn2, \
             tc.tile_pool(name="pmm6", bufs=3, space="PSUM") as pmm6:
            w1_sb = ffnw.tile([128, 8, D], BF16, tag="w1")
            w3_sb = ffnw.tile([128, 8, D], BF16, tag="w3")
            w2_sb = ffnw.tile([128, 8, D], BF16, tag="w2")
            nc.sync.dma_start(out=w1_sb, in_=w1T.rearrange("(c p) n -> p c n", p=128))
            nc.sync.dma_start(out=w3_sb, in_=w3T.rearrange("(c p) n -> p c n", p=128))
            nc.sync.dma_start(out=w2_sb, in_=w2T.rearrange("(c p) n -> p c n", p=128))
            gs = ffnw.tile([128, 8, ROWS_OUT], BF16, tag="gs")
            for f in range(8):
                for r2 in range(2):
                    ps = pmm6.tile([128, 512], FP32, tag="ps")
                    for c in range(8):
                        nc.tensor.matmul(
                            ps,
                            lhsT=w1_sb[:, c, f * 128:(f + 1) * 128],
                            rhs=h2g[r2][:, c, :],
                            start=(c == 0), stop=(c == 7),
                        )
                    us = ffn2.tile([128, 512], BF16, tag="us")
                    nc.scalar.activation(
                        out=us, in_=ps, func=AF.Silu, bias=b1_sb[:, f:f + 1],
                    )
                    ps2 = pmm6.tile([128, 512], FP32, tag="ps")
                    for c in range(8):
                        nc.tensor.matmul(
                            ps2,
                            lhsT=w3_sb[:, c, f * 128:(f + 1) * 128],
                            rhs=h2g[r2][:, c, :],
                            start=(c == 0), stop=(c == 7),
                        )
                    ts = ffn2.tile([128, 512], BF16, tag="ts")
                    nc.vector.tensor_scalar(
                        out=ts, in0=ps2, scalar1=b3_sb[:, f:f + 1], scalar2=None,
                        op0=mybir.AluOpType.add,
                    )
                    nc.vector.tensor_mul(gs[:, f, r2 * 512:(r2 + 1) * 512], us, ts)
            for rt in range(ROWS_OUT // 128):
                ysb = ffn2.tile([128, D], FP32, tag="ysb")
                for n in range(2):
                    ps = pmm6.tile([128, 512], FP32, tag="ps")
                    for f in range(8):
                        nc.tensor.matmul(
                            ps,
                            lhsT=gs[:, f, rt * 128:(rt + 1) * 128],
                            rhs=w2_sb[:, f, n * 512:(n + 1) * 512],
                            start=(f == 0), stop=(f == 7),
                        )
                    nc.vector.tensor_add(
                        ysb[:, n * 512:(n + 1) * 512], ps,
                        x2_sb[:, rt, n * 512:(n + 1) * 512],
                    )
                nc.sync.dma_start(out=y[rt * 128:(rt + 1) * 128, :], in_=ysb)

        p_x2_cm.__exit__(None, None, None)

    return nc

H = 16
_SLOPES = (2.0 ** (-8.0 / H)) ** np.arange(1, H + 1)
_RIDX = np.arange(S, dtype=np.float64)


def _prep_core_inputs(inp, core):
    b, half = core // 2, core % 2
    hsl = slice(half * DV, (half + 1) * DV)
    g1, b1 = f32(inp["ln1_g"]), f32(inp["ln1_b"])
    g2, b2 = f32(inp["ln2_g"]), f32(inp["ln2_b"])
    wq, wk, wv, wo = (f32(inp[k]) for k in ("wq", "wk", "wv", "wo"))
    w1, w2, w3 = (f32(inp[k]) for k in ("w1", "w2", "w3"))

    qkvbias = np.stack([
        (wq[hsl] @ b1) / 8.0,
        wk[hsl] @ b1,
        wv[hsl] @ b1,
    ]).astype(np.float32)
    b13 = np.stack([w1 @ b2, w3 @ b2]).astype(np.float32)

    qext = np.zeros((NH, 4, S), BFNP)
    kext = np.zeros((NH, 4, S), BFNP)
    for j in range(NH):
        s_hat = float(bf(np.float32(_SLOPES[half * NH + j])))
        qext[j, 0] = bf(np.full(S, s_hat))
        qext[j, 1] = bf(np.full(S, 256.0 * s_hat))
        qext[j, 2] = bf(-(_RIDX % 256))
        qext[j, 3] = bf(-(_RIDX // 256))
        kext[j, 0] = bf(_RIDX % 256)
        kext[j, 1] = bf(_RIDX // 256)
        kext[j, 2] = bf(np.full(S, s_hat))
        kext[j, 3] = bf(np.full(S, 256.0 * s_hat))

    out = {
        "x": f32(inp["x"][b]),
        "x_my": f32(inp["x"][b][half * ROWS_OUT:(half + 1) * ROWS_OUT]),
        "wqT": bf((wq[hsl] * g1[None, :]).T),
        "wkT": bf((wk[hsl] * g1[None, :]).T),
        "wvT": bf((wv[hsl] * g1[None, :]).T),
        "woT": bf(wo[:, hsl].T),
        "w1T": bf((w1 * g2[None, :]).T),
        "w3T": bf((w3 * g2[None, :]).T),
        "w2T": bf(w2.T),
        "qkvbias": qkvbias,
        "b13": b13,
        "qext": qext,
        "kext": kext,
    }
    return {k: np.ascontiguousarray(v) for k, v in out.items()}


_COMPILED_NC = None
LAST_EXEC_NS = None


def _get_nc():
    global _COMPILED_NC
    if _COMPILED_NC is None:
        nc = bacc.Bacc("TRN2", target_bir_lowering=False, debug=False,
                       num_devices=N_CORES)
        _build_kernel(nc)
        nc.compile()
        _COMPILED_NC = nc
    return _COMPILED_NC


def kernel(**inputs):
    global LAST_EXEC_NS
    nc = _get_nc()
    in_maps = [_prep_core_inputs(inputs, c) for c in range(N_CORES)]
    trace = os.environ.get("KERNEL_TRACE", "0") == "1"
    res = run_bass_kernel_spmd(nc, in_maps, list(range(N_CORES)), trace=trace)
    LAST_EXEC_NS = res.exec_time_ns
    B = inputs["x"].shape[0]
    y = np.zeros((B, S, D), np.float32)
    for core in range(N_CORES):
        b, half = core // 2, core % 2
        y[b, half * ROWS_OUT:(half + 1) * ROWS_OUT] = res.results[core]["y"]
    return y



# revision 3
# speedup vs baseline: 1.1838x; 1.1838x over previous
"""Trainium2 Bass kernel for nn_AttentionBlock (pre-LN MHA with ALiBi +
pre-LN SwiGLU FFN), distributed over 8 NeuronCores.

Sharding: core = (batch, head-parity). Each core computes LN1 + QKV +
attention for 8 of 16 heads over its batch's full 2048 rows, a partial
O-projection (its 512 of 1024 contraction dims), then pairwise
on-device ReduceScatters combine the partials and each core finishes
LN2 + SwiGLU FFN for half of its batch's rows.

ALiBi + causal handling: the additive alibi term s_hat*(c-r) is folded
into the score matmul via 4 augmentation rows (a c_lo/c_hi/r_lo/r_hi
split keeps every bf16 product exact); scores are computed transposed
[k, q], exponentiated without max subtraction (scores are bounded), the
causal diagonal is masked post-exp with affine_select, and the softmax
denominator comes from a ones-column appended to V.

Banding: alibi decay makes attention effectively banded per head —
exp(-s*d) < 2e-8 past d = 18/s, so k-tiles entirely beyond that band
are skipped (their weight is negligible vs the >= 1 softmax
denominator). Heads are assigned to the two cores of a batch in
adjacent pairs (2i, 2i+1) and slot i uses the wider band of the pair,
so both cores run the identical SPMD program with balanced work.

kernel(**inputs) takes the full unsharded inputs of
reference.setup_inputs() and returns the full (4, 2048, 1024) output.
"""

import os
from contextlib import ExitStack

import numpy as np
import ml_dtypes

import concourse.bass as bass
import concourse.bacc as bacc
import concourse.mybir as mybir
import concourse.tile as tile
from concourse.masks import make_identity
from concourse.bass_utils import run_bass_kernel_spmd

BFNP = ml_dtypes.bfloat16
bf = lambda a: np.asarray(a).astype(BFNP)
f32 = lambda a: np.asarray(a, np.float32)

FP32 = mybir.dt.float32
BF16 = mybir.dt.bfloat16
AF = mybir.ActivationFunctionType

S = 2048
N_CORES = 8
D = 1024          # model dim
NH = 8            # heads (slots) per core
DH = 64           # head dim
DV = NH * DH      # 512, per-core qkv dim
ROWS_OUT = 1024   # rows per core after ReduceScatter
NKT = S // 128    # 16 k-tiles
PAIRS = [[0, 1], [2, 3], [4, 5], [6, 7]]

# Per-slot attention band (k-columns). Slot i holds head 2i+half; the
# band is 18/slope of the smaller-slope head of the pair (2i+1),
# capped at S. Dropped weights are <= exp(-18) vs a denominator >= 1.
BAND = [36, 72, 144, 288, 576, 1152, 2048, 2048]


def _kt_lo(slot: int, qt: int) -> int:
    """First k-tile (128 wide) kept for 256-wide q-tile qt of slot."""
    a = qt * 256 - BAND[slot] - 127
    return max(0, -((-a) // 128))


def _build_kernel(nc):
    x = nc.dram_tensor("x", [S, D], FP32, kind="ExternalInput").ap()
    x_my = nc.dram_tensor("x_my", [ROWS_OUT, D], FP32, kind="ExternalInput").ap()
    wqT = nc.dram_tensor("wqT", [D, DV], BF16, kind="ExternalInput").ap()
    wkT = nc.dram_tensor("wkT", [D, DV], BF16, kind="ExternalInput").ap()
    wvT = nc.dram_tensor("wvT", [D, DV], BF16, kind="ExternalInput").ap()
    woT = nc.dram_tensor("woT", [DV, D], BF16, kind="ExternalInput").ap()
    w1T = nc.dram_tensor("w1T", [D, D], BF16, kind="ExternalInput").ap()
    w3T = nc.dram_tensor("w3T", [D, D], BF16, kind="ExternalInput").ap()
    w2T = nc.dram_tensor("w2T", [D, D], BF16, kind="ExternalInput").ap()
    qkvbias = nc.dram_tensor("qkvbias", [3, DV], FP32, kind="ExternalInput").ap()
    b13 = nc.dram_tensor("b13", [2, D], FP32, kind="ExternalInput").ap()
    qext = nc.dram_tensor("qext", [NH, 4, S], BF16, kind="ExternalInput").ap()
    kext = nc.dram_tensor("kext", [NH, 4, S], BF16, kind="ExternalInput").ap()

    y = nc.dram_tensor("y", [ROWS_OUT, D], FP32, kind="ExternalOutput").ap()
    with tile.TileContext(nc) as tc, ExitStack() as ctx:
        singles = ctx.enter_context(tc.tile_pool(name="singles", bufs=1))
        dram = ctx.enter_context(tc.tile_pool(name="dram", bufs=1, space="DRAM"))

        ident = singles.tile([128, 128], BF16)
        make_identity(nc, ident)

        eps = singles.tile([128, 1], FP32)
        nc.vector.memset(eps, 1e-5)

        # per-partition biases for Q/K evac ([128, 4] : dout-in-chunk x chunk)
        qb_sb = singles.tile([128, 4], FP32)
        kb_sb = singles.tile([128, 4], FP32)
        nc.sync.dma_start(out=qb_sb, in_=qkvbias[0].rearrange("(c p) -> p c", p=128))
        nc.sync.dma_start(out=kb_sb, in_=qkvbias[1].rearrange("(c p) -> p c", p=128))
        # V bias broadcast along partitions [128, 512]
        vb_sb = singles.tile([128, DV], FP32)
        nc.sync.dma_start(
            out=vb_sb,
            in_=bass.AP(tensor=qkvbias.tensor, offset=2 * DV, ap=[[0, 128], [1, DV]]),
        )
        b1_sb = singles.tile([128, 8], FP32)
        b3_sb = singles.tile([128, 8], FP32)
        nc.sync.dma_start(out=b1_sb, in_=b13[0].rearrange("(c p) -> p c", p=128))
        nc.sync.dma_start(out=b3_sb, in_=b13[1].rearrange("(c p) -> p c", p=128))

        # big tensors for phases 1-4 (one pool, closed after the O-proj)
        p_big_cm = tc.tile_pool(name="p_big", bufs=1)
        p_big = p_big_cm.__enter__()

        hTg = [p_big.tile([128, 8, 512], BF16, name=f"hT{g}", tag=f"hT{g}")
               for g in range(4)]                    # h^T row-groups of 512
        qaug = [p_big.tile([68, S], BF16, name=f"qaug{h}", tag=f"qaug{h}") for h in range(NH)]
        kaug = [p_big.tile([68, S], BF16, name=f"kaug{h}", tag=f"kaug{h}") for h in range(NH)]
        vsb = p_big.tile([128, NKT, NH * 65], BF16)  # v rows + ones col per head
        oT = p_big.tile([128, 4, S], BF16)
        o_rm = p_big.tile([128, NKT, DV], BF16)      # o row-major staging

        # ---------------- Phases 1+2 fused: LN1 + h^T + QKV, row-group interleaved ----------------
        with tc.tile_pool(name="ln1", bufs=4) as ln1p, \
             tc.tile_pool(name="xgp", bufs=3) as xgp, \
             tc.tile_pool(name="ptr1", bufs=4, space="PSUM") as ptr1, \
             tc.tile_pool(name="qkvw", bufs=1) as qkvw, \
             tc.tile_pool(name="pmm2", bufs=3, space="PSUM") as pmm2:
            wq_sb = qkvw.tile([128, 8, DV], BF16, tag="wq")
            wk_sb = qkvw.tile([128, 8, DV], BF16, tag="wk")
            wv_sb = qkvw.tile([128, 8, DV], BF16, tag="wv")
            nc.gpsimd.memset(vsb, 1.0)

            for g in range(4):
                xgs = []
                for gh in range(2):
                    xg = xgp.tile([128, 2, D], FP32, tag="xg",
                                  name=f"xg{g}_{gh}")
                    nc.sync.dma_start(
                        out=xg,
                        in_=x[g * 512 + gh * 256:g * 512 + gh * 256 + 256, :]
                            .rearrange("(a p) d -> p a d", p=128))
                    xgs.append(xg)
                for rt in range(4 * g, 4 * g + 4):
                    xt = xgs[(rt % 4) // 2][:, rt % 2, :]
                    stats = ln1p.tile([128, 2, 6], FP32, tag="stats")
                    xr = xt.rearrange("p (s f) -> p s f", s=2)
                    nc.vector.bn_stats(out=stats[:, 0, :], in_=xr[:, 0, :])
                    nc.vector.bn_stats(out=stats[:, 1, :], in_=xr[:, 1, :])
                    mv = ln1p.tile([128, 2], FP32, tag="mv")
                    nc.vector.bn_aggr(out=mv, in_=stats)
                    rstd = ln1p.tile([128, 1], FP32, tag="rstd")
                    nc.scalar.activation(out=rstd, in_=mv[:, 1:2], func=AF.Sqrt,
                                         bias=eps)
                    nc.vector.reciprocal(out=rstd, in_=rstd)
                    hrow = ln1p.tile([128, D], BF16, tag="hrow")
                    nc.vector.tensor_scalar(
                        out=hrow, in0=xt, scalar1=mv[:, 0:1], scalar2=rstd,
                        op0=mybir.AluOpType.subtract, op1=mybir.AluOpType.mult,
                    )
                    for c in range(8):
                        pt = ptr1.tile([128, 128], BF16, tag="pt")
                        nc.tensor.transpose(pt, hrow[:, c * 128:(c + 1) * 128], ident)
                        nc.vector.tensor_copy(
                            out=hTg[rt // 4][:, c, (rt % 4) * 128:(rt % 4) * 128 + 128],
                            in_=pt)

                if g == 0:
                    nc.gpsimd.dma_start(out=wq_sb, in_=wqT.rearrange("(c p) n -> p c n", p=128))
                    nc.gpsimd.dma_start(out=wk_sb, in_=wkT.rearrange("(c p) n -> p c n", p=128))
                    nc.gpsimd.dma_start(out=wv_sb, in_=wvT.rearrange("(c p) n -> p c n", p=128))
                    for h in range(NH):
                        nc.gpsimd.dma_start(out=qaug[h][64:68, :], in_=qext[h])
                        nc.gpsimd.dma_start(out=kaug[h][64:68, :], in_=kext[h])
                # Q/K for this row group (Q evac on DVE, K evac on ACT)
                for (w_sb, aug, bias, scale) in (
                    (wq_sb, qaug, qb_sb, 0.125),
                    (wk_sb, kaug, kb_sb, None),
                ):
                    for m in range(4):
                        ps = pmm2.tile([128, 512], FP32, tag="ps")
                        for c in range(8):
                            nc.tensor.matmul(
                                ps,
                                lhsT=w_sb[:, c, m * 128:(m + 1) * 128],
                                rhs=hTg[g][:, c, :],
                                start=(c == 0), stop=(c == 7),
                            )
                        for sub in range(2):
                            if scale is not None:
                                nc.vector.tensor_scalar(
                                    out=aug[2 * m + sub][0:64, g * 512:(g + 1) * 512],
                                    in0=ps[sub * 64:(sub + 1) * 64, :],
                                    scalar1=scale,
                                    scalar2=bias[sub * 64:(sub + 1) * 64, m:m + 1],
                                    op0=mybir.AluOpType.mult,
                                    op1=mybir.AluOpType.add,
                                )
                            else:
                                nc.scalar.activation(
                                    out=aug[2 * m + sub][0:64, g * 512:(g + 1) * 512],
                                    in_=ps[sub * 64:(sub + 1) * 64, :],
                                    func=AF.Identity,
                                    bias=bias[sub * 64:(sub + 1) * 64, m:m + 1],
                                    scale=1.0,
                                )
                # V for this row group (row-major out)
                for kt in range(4 * g, 4 * g + 4):
                    ps = pmm2.tile([128, 512], FP32, tag="ps")
                    for c in range(8):
                        nc.tensor.matmul(
                            ps,
                            lhsT=hTg[kt // 4][:, c, (kt % 4) * 128:(kt % 4) * 128 + 128],
                            rhs=wv_sb[:, c, :],
                            start=(c == 0), stop=(c == 7),
                        )
                    out_ap = vsb[:, kt, :].rearrange("p (h e) -> p h e", h=NH)[:, :, 0:64]
                    in_ap = ps.rearrange("p (h e) -> p h e", h=NH)
                    nc.vector.scalar_tensor_tensor(
                        out=out_ap, in0=in_ap, scalar=1.0,
                        in1=vb_sb.rearrange("p (h e) -> p h e", h=NH),
                        op0=mybir.AluOpType.mult, op1=mybir.AluOpType.add,
                    )

        # ---------------- Phase 3: banded attention (head pairs interleaved) ----------------
        with tc.tile_pool(name="att", bufs=4) as attp, \
             tc.tile_pool(name="psc", bufs=2, space="PSUM") as psc, \
             tc.tile_pool(name="poa", bufs=4, space="PSUM") as poa:
            for hp in range(NH // 2):
                heads = (2 * hp, 2 * hp + 1)
                for qt in range(8):          # 256-wide q tiles
                    kt_hi = 2 * qt + 1       # causal upper k-tile
                    lo = [_kt_lo(h, qt) for h in heads]
                    oacc = [poa.tile([128, 65], FP32, name=f"oa{hp}_{qt}_{ii}",
                                     tag="oacc") for ii in range(4)]
                    for chunk in range(4):   # at most 4 chunks of 4 k-tiles
                        ats = []
                        spans = []
                        for idx, h in enumerate(heads):
                            g0 = lo[idx] + 4 * chunk
                            if g0 > kt_hi:
                                ats.append(None)
                                spans.append(None)
                                continue
                            w = min(4, kt_hi + 1 - g0)
                            sc = psc.tile([128, 1024], FP32, tag="sc",
                                          name=f"sc{hp}_{qt}_{chunk}_{idx}")
                            for i in range(w):
                                kt = g0 + i
                                nc.tensor.matmul(
                                    sc[:, i * 256:(i + 1) * 256],
                                    lhsT=kaug[h][:, kt * 128:(kt + 1) * 128],
                                    rhs=qaug[h][:, qt * 256:(qt + 1) * 256],
                                    start=True, stop=True,
                                )
                            at = attp.tile([128, 1024], BF16, tag="at",
                                           name=f"at{hp}_{qt}_{chunk}_{idx}")
                            nc.scalar.activation(
                                out=at[:, :w * 256], in_=sc[:, :w * 256], func=AF.Exp,
                            )
                            for i in range(w):
                                kt = g0 + i
                                if kt >= 2 * qt:  # diagonal tiles: mask c > r
                                    nc.gpsimd.affine_select(
                                        out=at[:, i * 256:(i + 1) * 256],
                                        in_=at[:, i * 256:(i + 1) * 256],
                                        compare_op=mybir.AluOpType.is_ge,
                                        fill=0.0,
                                        base=qt * 256 - kt * 128,
                                        channel_multiplier=-1,
                                        pattern=[[1, 256]],
                                    )
                            ats.append(at)
                            spans.append((g0, w))
                        for idx, h in enumerate(heads):
                            if ats[idx] is None:
                                continue
                            at = ats[idx]
                            g0, w = spans[idx]
                            for sub in range(2):
                                for i in range(w):
                                    kt = g0 + i
                                    nc.tensor.matmul(
                                        oacc[2 * idx + sub],
                                        lhsT=at[:, i * 256 + sub * 128:
                                                i * 256 + sub * 128 + 128],
                                        rhs=vsb[:, kt, h * 65:(h + 1) * 65],
                                        start=(kt == lo[idx]), stop=(kt == kt_hi),
                                    )
                    for idx, h in enumerate(heads):
                        for sub in range(2):
                            oa = oacc[2 * idx + sub]
                            rec = attp.tile([128, 1], FP32, tag="rec")
                            nc.vector.reciprocal(out=rec, in_=oa[:, 64:65])
                            rr = qt * 2 + sub
                            nc.vector.tensor_scalar_mul(
                                out=o_rm[:, rr, h * 64:(h + 1) * 64],
                                in0=oa[:, 0:64], scalar1=rec,
                            )

        # Phase 3.5: transpose o to feature-major
        with tc.tile_pool(name="ptr35", bufs=4, space="PSUM") as ptr35:
            for rr in range(NKT):
                for c in range(4):
                    pt = ptr35.tile([128, 128], BF16, tag="pt")
                    nc.tensor.transpose(pt, o_rm[:, rr, c * 128:(c + 1) * 128], ident)
                    nc.vector.tensor_copy(out=oT[:, c, rr * 128:(rr + 1) * 128], in_=pt)

        # ---------------- Phase 4: O-proj + split ReduceScatter ----------------
        # Two half-size ReduceScatters so the first one overlaps the
        # second half of the O-projection and phase 5 overlaps the second.
        cc_in = [dram.tile([ROWS_OUT, D], BF16, name=f"ccin{i}") for i in range(2)]
        cc_out = [dram.tile([ROWS_OUT // 2, D], BF16, name=f"ccout{i}") for i in range(2)]
        # rt tiles feeding collective 0: own-half rows 0-511 + peer rows
        RT_ORDER = [[0, 1, 2, 3, 8, 9, 10, 11], [4, 5, 6, 7, 12, 13, 14, 15]]
        with tc.tile_pool(name="wop", bufs=1) as wop, \
             tc.tile_pool(name="oproj", bufs=3) as op, \
             tc.tile_pool(name="pmm4", bufs=3, space="PSUM") as pmm4:
            wo_sb = wop.tile([128, 4, D], BF16, tag="wo")
            nc.sync.dma_start(out=wo_sb, in_=woT.rearrange("(c p) n -> p c n", p=128))
            for half in range(2):
                for j, rt in enumerate(RT_ORDER[half]):
                    row_sb = op.tile([128, D], BF16, tag="row")
                    for n in range(2):
                        ps = pmm4.tile([128, 512], FP32, tag="ps")
                        for c in range(4):
                            nc.tensor.matmul(
                                ps,
                                lhsT=oT[:, c, rt * 128:(rt + 1) * 128],
                                rhs=wo_sb[:, c, n * 512:(n + 1) * 512],
                                start=(c == 0), stop=(c == 3),
                            )
                        nc.vector.tensor_copy(out=row_sb[:, n * 512:(n + 1) * 512], in_=ps)
                    nc.sync.dma_start(
                        out=cc_in[half][j * 128:(j + 1) * 128, :], in_=row_sb)
                nc.gpsimd.collective_compute(
                    "ReduceScatter",
                    mybir.AluOpType.add,
                    ins=[cc_in[half].opt()],
                    outs=[cc_out[half].opt()],
                    replica_groups=PAIRS,
                )

        p_big_cm.__exit__(None, None, None)
        p_x2_cm = tc.tile_pool(name="p_x2", bufs=1)          # phases 5-6
        p_x2 = p_x2_cm.__enter__()
        x2_sb = p_x2.tile([128, 8, D], FP32)
        h2g = [p_x2.tile([128, 8, 512], BF16, name=f"h2T{g}", tag=f"h2T{g}")
               for g in range(2)]

        # ---------------- Phase 5: x2 + LN2 + h2^T ----------------
        with tc.tile_pool(name="ln2", bufs=3) as ln2p, \
             tc.tile_pool(name="ptr5", bufs=4, space="PSUM") as ptr5:
            for rt in range(ROWS_OUT // 128):
                xt = ln2p.tile([128, D], FP32, tag="xt")
                nc.sync.dma_start(out=xt, in_=x_my[rt * 128:(rt + 1) * 128, :])
                rs = ln2p.tile([128, D], BF16, tag="rs")
                nc.sync.dma_start(
                    out=rs,
                    in_=cc_out[rt // 4][(rt % 4) * 128:(rt % 4) * 128 + 128, :])
                nc.vector.tensor_add(x2_sb[:, rt, :], xt, rs)
                stats = ln2p.tile([128, 2, 6], FP32, tag="stats")
                x2r = x2_sb[:, rt, :].rearrange("p (s f) -> p s f", s=2)
                nc.vector.bn_stats(out=stats[:, 0, :], in_=x2r[:, 0, :])
                nc.vector.bn_stats(out=stats[:, 1, :], in_=x2r[:, 1, :])
                mv = ln2p.tile([128, 2], FP32, tag="mv")
                nc.vector.bn_aggr(out=mv, in_=stats)
                rstd = ln2p.tile([128, 1], FP32, tag="rstd")
                nc.scalar.activation(out=rstd, in_=mv[:, 1:2], func=AF.Sqrt, bias=eps)
                nc.vector.reciprocal(out=rstd, in_=rstd)
                hrow = ln2p.tile([128, D], BF16, tag="hrow")
                nc.vector.tensor_scalar(
                    out=hrow, in0=x2_sb[:, rt, :], scalar1=mv[:, 0:1], scalar2=rstd,
                    op0=mybir.AluOpType.subtract, op1=mybir.AluOpType.mult,
                )
                for c in range(8):
                    pt = ptr5.tile([128, 128], BF16, tag="pt")
                    nc.tensor.transpose(pt, hrow[:, c * 128:(c + 1) * 128], ident)
                    nc.vector.tensor_copy(
                        out=h2g[rt // 4][:, c, (rt % 4) * 128:(rt % 4) * 128 + 128],
                        in_=pt)

        # ---------------- Phase 6: FFN ----------------
        with tc.tile_pool(name="ffnw", bufs=1) as ffnw, \
             tc.tile_pool(name="ffn2", bufs=3) as ffn2, \
             tc.tile_pool(name="pmm6", bufs=3, space="PSUM") as pmm6:
            w1_sb = ffnw.tile([128, 8, D], BF16, tag="w1")
            w3_sb = ffnw.tile([128, 8, D], BF16, tag="w3")
            w2_sb = ffnw.tile([128, 8, D], BF16, tag="w2")
            nc.sync.dma_start(out=w1_sb, in_=w1T.rearrange("(c p) n -> p c n", p=128))
            nc.sync.dma_start(out=w3_sb, in_=w3T.rearrange("(c p) n -> p c n", p=128))
            nc.sync.dma_start(out=w2_sb, in_=w2T.rearrange("(c p) n -> p c n", p=128))
            gs = ffnw.tile([128, 8, ROWS_OUT], BF16, tag="gs")
            for f in range(8):
                for r2 in range(2):
                    ps = pmm6.tile([128, 512], FP32, tag="ps")
                    for c in range(8):
                        nc.tensor.matmul(
                            ps,
                            lhsT=w1_sb[:, c, f * 128:(f + 1) * 128],
                            rhs=h2g[r2][:, c, :],
                            start=(c == 0), stop=(c == 7),
                        )
                    us = ffn2.tile([128, 512], BF16, tag="us")
                    nc.scalar.activation(
                        out=us, in_=ps, func=AF.Silu, bias=b1_sb[:, f:f + 1],
                    )
                    ps2 = pmm6.tile([128, 512], FP32, tag="ps")
                    for c in range(8):
                        nc.tensor.matmul(
                            ps2,
                            lhsT=w3_sb[:, c, f * 128:(f + 1) * 128],
                            rhs=h2g[r2][:, c, :],
                            start=(c == 0), stop=(c == 7),
                        )
                    ts = ffn2.tile([128, 512], BF16, tag="ts")
                    nc.vector.tensor_scalar(
                        out=ts, in0=ps2, scalar1=b3_sb[:, f:f + 1], scalar2=None,
                        op0=mybir.AluOpType.add,
                    )
                    nc.vector.tensor_mul(gs[:, f, r2 * 512:(r2 + 1) * 512], us, ts)
            for rt in range(ROWS_OUT // 128):
                ysb = ffn2.tile([128, D], FP32, tag="ysb")
                for n in range(2):
                    ps = pmm6.tile([128, 512], FP32, tag="ps")
                    for f in range(8):
                        nc.tensor.matmul(
                            ps,
                            lhsT=gs[:, f, rt * 128:(rt + 1) * 128],
                            rhs=w2_sb[:, f, n * 512:(n + 1) * 512],
                            start=(f == 0), stop=(f == 7),
                        )
                    nc.vector.tensor_add(
                        ysb[:, n * 512:(n + 1) * 512], ps,
                        x2_sb[:, rt, n * 512:(n + 1) * 512],
                    )
                nc.sync.dma_start(out=y[rt * 128:(rt + 1) * 128, :], in_=ysb)

        p_x2_cm.__exit__(None, None, None)

    return nc

H = 16
_SLOPES = (2.0 ** (-8.0 / H)) ** np.arange(1, H + 1)
_RIDX = np.arange(S, dtype=np.float64)


def _prep_core_inputs(inp, core):
    b, half = core // 2, core % 2
    # slot i of this core holds head 2i+half (0-indexed)
    head_ids = [2 * i + half for i in range(NH)]
    dims = np.concatenate([np.arange(64 * h, 64 * h + 64) for h in head_ids])
    g1, b1 = f32(inp["ln1_g"]), f32(inp["ln1_b"])
    g2, b2 = f32(inp["ln2_g"]), f32(inp["ln2_b"])
    wq, wk, wv, wo = (f32(inp[k]) for k in ("wq", "wk", "wv", "wo"))
    w1, w2, w3 = (f32(inp[k]) for k in ("w1", "w2", "w3"))

    qkvbias = np.stack([
        (wq[dims] @ b1) / 8.0,
        wk[dims] @ b1,
        wv[dims] @ b1,
    ]).astype(np.float32)
    b13 = np.stack([w1 @ b2, w3 @ b2]).astype(np.float32)

    qext = np.zeros((NH, 4, S), BFNP)
    kext = np.zeros((NH, 4, S), BFNP)
    for j in range(NH):
        s_hat = float(bf(np.float32(_SLOPES[head_ids[j]])))
        qext[j, 0] = bf(np.full(S, s_hat))
        qext[j, 1] = bf(np.full(S, 256.0 * s_hat))
        qext[j, 2] = bf(-(_RIDX % 256))
        qext[j, 3] = bf(-(_RIDX // 256))
        kext[j, 0] = bf(_RIDX % 256)
        kext[j, 1] = bf(_RIDX // 256)
        kext[j, 2] = bf(np.full(S, s_hat))
        kext[j, 3] = bf(np.full(S, 256.0 * s_hat))

    out = {
        "x": f32(inp["x"][b]),
        "x_my": f32(inp["x"][b][half * ROWS_OUT:(half + 1) * ROWS_OUT]),
        "wqT": bf((wq[dims] * g1[None, :]).T),
        "wkT": bf((wk[dims] * g1[None, :]).T),
        "wvT": bf((wv[dims] * g1[None, :]).T),
        "woT": bf(wo[:, dims].T),
        "w1T": bf((w1 * g2[None, :]).T),
        "w3T": bf((w3 * g2[None, :]).T),
        "w2T": bf(w2.T),
        "qkvbias": qkvbias,
        "b13": b13,
        "qext": qext,
        "kext": kext,
    }
    return {k: np.ascontiguousarray(v) for k, v in out.items()}


_COMPILED_NC = None
LAST_EXEC_NS = None


def _get_nc():
    global _COMPILED_NC
    if _COMPILED_NC is None:
        nc = bacc.Bacc("TRN2", target_bir_lowering=False, debug=False,
                       num_devices=N_CORES)
        _build_kernel(nc)
        nc.compile()
        _COMPILED_NC = nc
    return _COMPILED_NC


def kernel(**inputs):
    global LAST_EXEC_NS
    nc = _get_nc()
    in_maps = [_prep_core_inputs(inputs, c) for c in range(N_CORES)]
    trace = os.environ.get("KERNEL_TRACE", "0") == "1"
    res = run_bass_kernel_spmd(nc, in_maps, list(range(N_CORES)), trace=trace)
    LAST_EXEC_NS = res.exec_time_ns
    B = inputs["x"].shape[0]
    y = np.zeros((B, S, D), np.float32)
    for core in range(N_CORES):
        b, half = core // 2, core % 2
        y[b, half * ROWS_OUT:(half + 1) * ROWS_OUT] = res.results[core]["y"]
    return y


# revision 11
# speedup vs baseline: 1.3916x; 1.1755x over previous
"""Trainium2 Bass kernel for nn_AttentionBlock (pre-LN MHA with ALiBi +
pre-LN SwiGLU FFN), distributed over 8 NeuronCores.

Sharding: core = (batch, head-parity). Each core computes LN1 + QKV +
banded attention for 8 of 16 heads over its batch's full 2048 rows, a
partial O-projection (its 512 of 1024 contraction dims) folded into the
attention loop per q-tile, then two pairwise ReduceScatters (fired at
40% / 100% of attention so they hide behind compute) combine the
partials; each core finishes LN2 + SwiGLU FFN for half of its batch's
rows, with the first FFN half overlapping the second collective.

ALiBi + causal handling: the additive alibi term s_hat*(c-r) is folded
into the score matmul via 4 augmentation rows (a c_lo/c_hi/r_lo/r_hi
split keeps every bf16 product exact); scores are computed transposed
[k, q], exponentiated without max subtraction (scores are bounded), the
causal diagonal is masked post-exp with affine_select, and the softmax
denominator comes from a ones-column appended to V.

Banding: alibi decay makes attention effectively banded per head —
exp(-s*d) < 4e-7 past d = 15/s, so k-tiles entirely beyond that band
are skipped (their weight is negligible vs the >= 1 softmax
denominator). Heads are assigned to the two cores of a batch in
adjacent pairs (2i, 2i+1) and slot i uses the wider band of the pair,
so both cores run the identical SPMD program with balanced work.

kernel(**inputs) takes the full unsharded inputs of
reference.setup_inputs() and returns the full (4, 2048, 1024) output.
"""

import os
from contextlib import ExitStack

import numpy as np
import ml_dtypes

import concourse.bass as bass
import concourse.bacc as bacc
import concourse.mybir as mybir
import concourse.tile as tile
from concourse.masks import make_identity
from concourse.bass_utils import run_bass_kernel_spmd

BFNP = ml_dtypes.bfloat16
bf = lambda a: np.asarray(a).astype(BFNP)
f32 = lambda a: np.asarray(a, np.float32)

FP32 = mybir.dt.float32
BF16 = mybir.dt.bfloat16
AF = mybir.ActivationFunctionType

S = 2048
N_CORES = 8
D = 1024          # model dim
NH = 8            # heads (slots) per core
DH = 64           # head dim
DV = NH * DH      # 512, per-core qkv dim
ROWS_OUT = 1024   # rows per core after ReduceScatter
NKT = S // 128    # 16 k-tiles
PAIRS = [[0, 1], [2, 3], [4, 5], [6, 7]]

# Per-slot attention band (k-columns). Slot i holds head 2i+half; the
# band is 15/slope of the smaller-slope head of the pair (2i+1),
# capped at S. Dropped weights are <= exp(-15) vs a denominator >= 1.
BAND = [30, 60, 120, 240, 480, 960, 1920, 2048]

# q-tile processing order: the first four q-tiles fill collective 0
# (each core's first 512 output rows), the rest fill collective 1.
QT_GROUPS = [[0, 1, 4, 5], [2, 3, 6, 7]]


def _kt_lo(slot: int, qt: int) -> int:
    """First k-tile (128 wide) kept for 256-wide q-tile qt of slot."""
    a = qt * 256 - BAND[slot] - 127
    return max(0, -((-a) // 128))


def _build_kernel(nc):
    x = nc.dram_tensor("x", [S, D], FP32, kind="ExternalInput").ap()
    x_my = nc.dram_tensor("x_my", [ROWS_OUT, D], FP32, kind="ExternalInput").ap()
    wqT = nc.dram_tensor("wqT", [D, DV], BF16, kind="ExternalInput").ap()
    wkT = nc.dram_tensor("wkT", [D, DV], BF16, kind="ExternalInput").ap()
    wvT = nc.dram_tensor("wvT", [D, DV], BF16, kind="ExternalInput").ap()
    woT = nc.dram_tensor("woT", [DV, D], BF16, kind="ExternalInput").ap()
    w1T = nc.dram_tensor("w1T", [D, D], BF16, kind="ExternalInput").ap()
    w3T = nc.dram_tensor("w3T", [D, D], BF16, kind="ExternalInput").ap()
    w2T = nc.dram_tensor("w2T", [D, D], BF16, kind="ExternalInput").ap()
    qkvbias = nc.dram_tensor("qkvbias", [3, DV], FP32, kind="ExternalInput").ap()
    b13 = nc.dram_tensor("b13", [2, D], FP32, kind="ExternalInput").ap()
    qext = nc.dram_tensor("qext", [NH, 4, S], BF16, kind="ExternalInput").ap()
    kext = nc.dram_tensor("kext", [NH, 4, S], BF16, kind="ExternalInput").ap()

    y = nc.dram_tensor("y", [ROWS_OUT, D], FP32, kind="ExternalOutput").ap()
    with tile.TileContext(nc) as tc, ExitStack() as ctx:
        singles = ctx.enter_context(tc.tile_pool(name="singles", bufs=1))
        dram = ctx.enter_context(tc.tile_pool(name="dram", bufs=1, space="DRAM"))
        wop = ctx.enter_context(tc.tile_pool(name="wop", bufs=1))
        wo_sb = wop.tile([128, 4, D], BF16, tag="wo")
        w13p = ctx.enter_context(tc.tile_pool(name="w13p", bufs=1))
        w1_sb = w13p.tile([128, 8, D], BF16, tag="w1")
        w3_sb = w13p.tile([128, 8, D], BF16, tag="w3")

        ident = singles.tile([128, 128], BF16)
        make_identity(nc, ident)

        eps = singles.tile([128, 1], FP32)
        nc.vector.memset(eps, 1e-5)

        # per-partition biases for Q/K evac ([128, 4] : dout-in-chunk x chunk)
        qb_sb = singles.tile([128, 4], FP32)
        kb_sb = singles.tile([128, 4], FP32)
        nc.sync.dma_start(out=qb_sb, in_=qkvbias[0].rearrange("(c p) -> p c", p=128))
        nc.sync.dma_start(out=kb_sb, in_=qkvbias[1].rearrange("(c p) -> p c", p=128))
        # V bias broadcast along partitions [128, 512]
        vb_sb = singles.tile([128, DV], FP32)
        nc.sync.dma_start(
            out=vb_sb,
            in_=bass.AP(tensor=qkvbias.tensor, offset=2 * DV, ap=[[0, 128], [1, DV]]),
        )
        b1_sb = singles.tile([128, 8], FP32)
        b3_sb = singles.tile([128, 8], FP32)
        nc.sync.dma_start(out=b1_sb, in_=b13[0].rearrange("(c p) -> p c", p=128))
        nc.sync.dma_start(out=b3_sb, in_=b13[1].rearrange("(c p) -> p c", p=128))

        # big tensors for phases 1-4 (one pool, closed after the O-proj)
        p_big_cm = tc.tile_pool(name="p_big", bufs=1)
        p_big = p_big_cm.__enter__()
        qaug = [p_big.tile([68, S], BF16, name=f"qaug{h}", tag=f"qaug{h}") for h in range(NH)]
        kaug = [p_big.tile([68, S], BF16, name=f"kaug{h}", tag=f"kaug{h}") for h in range(NH)]
        vsb = p_big.tile([128, NKT, NH * 65], BF16)  # v rows + ones col per head

        # h^T row-groups, freed after the QKV matmuls
        p_hT_cm = tc.tile_pool(name="p_hT", bufs=1)
        p_hT = p_hT_cm.__enter__()
        hTg = [p_hT.tile([128, 8, 512], BF16, name=f"hT{g}", tag=f"hT{g}")
               for g in range(4)]

        # ---------------- Phases 1+2 fused: LN1 + h^T + QKV, row-group interleaved ----------------
        with tc.tile_pool(name="ln1", bufs=4) as ln1p, \
             tc.tile_pool(name="xgp", bufs=4) as xgp, \
             tc.tile_pool(name="ptr1", bufs=4, space="PSUM") as ptr1, \
             tc.tile_pool(name="qkvw", bufs=1) as qkvw, \
             tc.tile_pool(name="pmm2", bufs=3, space="PSUM") as pmm2:
            wq_sb = qkvw.tile([128, 8, DV], BF16, tag="wq")
            wk_sb = qkvw.tile([128, 8, DV], BF16, tag="wk")
            wv_sb = qkvw.tile([128, 8, DV], BF16, tag="wv")
            nc.gpsimd.memset(vsb, 1.0)

            for g in range(4):
                xgs = []
                for gh in range(4):
                    xg = xgp.tile([128, D], FP32, tag="xg",
                                  name=f"xg{g}_{gh}")
                    nc.sync.dma_start(
                        out=xg,
                        in_=x[g * 512 + gh * 128:g * 512 + gh * 128 + 128, :])
                    xgs.append(xg)
                for rt in range(4 * g, 4 * g + 4):
                    xt = xgs[rt % 4]
                    stats = ln1p.tile([128, 2, 6], FP32, tag="stats")
                    xr = xt.rearrange("p (s f) -> p s f", s=2)
                    nc.vector.bn_stats(out=stats[:, 0, :], in_=xr[:, 0, :])
                    nc.vector.bn_stats(out=stats[:, 1, :], in_=xr[:, 1, :])
                    mv = ln1p.tile([128, 2], FP32, tag="mv")
                    nc.vector.bn_aggr(out=mv, in_=stats)
                    rstd = ln1p.tile([128, 1], FP32, tag="rstd")
                    nc.scalar.activation(out=rstd, in_=mv[:, 1:2], func=AF.Sqrt,
                                         bias=eps)
                    nc.vector.reciprocal(out=rstd, in_=rstd)
                    hrow = ln1p.tile([128, D], BF16, tag="hrow")
                    nc.vector.tensor_scalar(
                        out=hrow, in0=xt, scalar1=mv[:, 0:1], scalar2=rstd,
                        op0=mybir.AluOpType.subtract, op1=mybir.AluOpType.mult,
                    )
                    for c in range(8):
                        pt = ptr1.tile([128, 128], BF16, tag="pt")
                        nc.tensor.transpose(pt, hrow[:, c * 128:(c + 1) * 128], ident)
                        nc.vector.tensor_copy(
                            out=hTg[rt // 4][:, c, (rt % 4) * 128:(rt % 4) * 128 + 128],
                            in_=pt)

                if g == 0:
                    nc.gpsimd.dma_start(out=wq_sb, in_=wqT.rearrange("(c p) n -> p c n", p=128))
                    nc.gpsimd.dma_start(out=wk_sb, in_=wkT.rearrange("(c p) n -> p c n", p=128))
                    nc.gpsimd.dma_start(out=wv_sb, in_=wvT.rearrange("(c p) n -> p c n", p=128))
                    for h in range(NH):
                        nc.gpsimd.dma_start(out=qaug[h][64:68, :], in_=qext[h])
                        nc.gpsimd.dma_start(out=kaug[h][64:68, :], in_=kext[h])
                # Q/K for this row group (Q evac on DVE, K evac on ACT)
                for (w_sb, aug, bias, scale) in (
                    (wq_sb, qaug, qb_sb, 0.125),
                    (wk_sb, kaug, kb_sb, None),
                ):
                    for m in range(4):
                        ps = pmm2.tile([128, 512], FP32, tag="ps")
                        for c in range(8):
                            nc.tensor.matmul(
                                ps,
                                lhsT=w_sb[:, c, m * 128:(m + 1) * 128],
                                rhs=hTg[g][:, c, :],
                                start=(c == 0), stop=(c == 7),
                            )
                        for sub in range(2):
                            if scale is not None:
                                nc.vector.tensor_scalar(
                                    out=aug[2 * m + sub][0:64, g * 512:(g + 1) * 512],
                                    in0=ps[sub * 64:(sub + 1) * 64, :],
                                    scalar1=scale,
                                    scalar2=bias[sub * 64:(sub + 1) * 64, m:m + 1],
                                    op0=mybir.AluOpType.mult,
                                    op1=mybir.AluOpType.add,
                                )
                            else:
                                nc.scalar.activation(
                                    out=aug[2 * m + sub][0:64, g * 512:(g + 1) * 512],
                                    in_=ps[sub * 64:(sub + 1) * 64, :],
                                    func=AF.Identity,
                                    bias=bias[sub * 64:(sub + 1) * 64, m:m + 1],
                                    scale=1.0,
                                )
                # V for this row group (row-major out)
                for kt in range(4 * g, 4 * g + 4):
                    ps = pmm2.tile([128, 512], FP32, tag="ps")
                    for c in range(8):
                        nc.tensor.matmul(
                            ps,
                            lhsT=hTg[kt // 4][:, c, (kt % 4) * 128:(kt % 4) * 128 + 128],
                            rhs=wv_sb[:, c, :],
                            start=(c == 0), stop=(c == 7),
                        )
                    out_ap = vsb[:, kt, :].rearrange("p (h e) -> p h e", h=NH)[:, :, 0:64]
                    in_ap = ps.rearrange("p (h e) -> p h e", h=NH)
                    nc.vector.scalar_tensor_tensor(
                        out=out_ap, in0=in_ap, scalar=1.0,
                        in1=vb_sb.rearrange("p (h e) -> p h e", h=NH),
                        op0=mybir.AluOpType.mult, op1=mybir.AluOpType.add,
                    )

        p_hT_cm.__exit__(None, None, None)

        # O-proj weights + FFN gate/up weights prefetch (overlap attention)
        nc.sync.dma_start(out=wo_sb, in_=woT.rearrange("(c p) n -> p c n", p=128))
        nc.sync.dma_start(out=w1_sb, in_=w1T.rearrange("(c p) n -> p c n", p=128))
        nc.sync.dma_start(out=w3_sb, in_=w3T.rearrange("(c p) n -> p c n", p=128))

        cc_in = [dram.tile([ROWS_OUT, D], BF16, name=f"ccin{i}") for i in range(2)]
        cc_out = [dram.tile([ROWS_OUT // 2, D], BF16, name=f"ccout{i}") for i in range(2)]

        # ---------------- Phase 3+4: banded attention with O-proj folded in ----------------
        with tc.tile_pool(name="p_orm", bufs=1) as p_orm, \
             tc.tile_pool(name="att", bufs=10) as attp, \
             tc.tile_pool(name="psc", bufs=2, space="PSUM") as psc, \
             tc.tile_pool(name="poa", bufs=2, space="PSUM") as poa, \
             tc.tile_pool(name="ptro", bufs=1, space="PSUM") as ptro, \
             tc.tile_pool(name="pmm4", bufs=1, space="PSUM") as pmm4, \
             tc.tile_pool(name="oproj", bufs=2) as op:
            o_rm = p_orm.tile([128, NKT, DV], BF16)  # o row-major staging
            for cchalf in range(2):
                for gpos, qt in enumerate(QT_GROUPS[cchalf]):
                    kt_hi = 2 * qt + 1       # causal upper k-tile
                    for hp in range(NH // 2):
                        heads = (2 * hp, 2 * hp + 1)
                        lo = [_kt_lo(h, qt) for h in heads]
                        oacc = poa.tile([128, 4, 65], FP32, tag="oacc",
                                        name=f"oa{qt}_{hp}")
                        # scores + exp for all chunks (at tiles buffered)
                        head_ats = [[], []]
                        for chunk in range(4):   # chunks of up to 4 k-tiles
                            for idx, h in enumerate(heads):
                                g0 = lo[idx] + 4 * chunk
                                if g0 > kt_hi:
                                    continue
                                w = min(4, kt_hi + 1 - g0)
                                sc = psc.tile([128, 1024], FP32, tag="sc",
                                              name=f"sc{qt}_{hp}_{chunk}_{idx}")
                                for i in range(w):
                                    kt = g0 + i
                                    nc.tensor.matmul(
                                        sc[:, i * 256:(i + 1) * 256],
                                        lhsT=kaug[h][:, kt * 128:(kt + 1) * 128],
                                        rhs=qaug[h][:, qt * 256:(qt + 1) * 256],
                                        start=True, stop=True,
                                    )
                                at = attp.tile([128, 1024], BF16, tag="at",
                                               name=f"at{qt}_{hp}_{chunk}_{idx}")
                                nc.scalar.activation(
                                    out=at[:, :w * 256], in_=sc[:, :w * 256],
                                    func=AF.Exp,
                                )
                                for i in range(w):
                                    kt = g0 + i
                                    if kt >= 2 * qt:  # diagonal tiles: mask c > r
                                        nc.gpsimd.affine_select(
                                            out=at[:, i * 256:(i + 1) * 256],
                                            in_=at[:, i * 256:(i + 1) * 256],
                                            compare_op=mybir.AluOpType.is_ge,
                                            fill=0.0,
                                            base=qt * 256 - kt * 128,
                                            channel_multiplier=-1,
                                            pattern=[[1, 256]],
                                        )
                                head_ats[idx].append((at, g0, w))
                        # A-V accumulation: each (head, sub) group runs
                        # start->stop contiguously — a group's start clears
                        # has_written for the WHOLE psum bank, so groups
                        # sharing the oacc bank must not interleave.
                        for idx, h in enumerate(heads):
                            for sub in range(2):
                                for at, g0, w in head_ats[idx]:
                                    for i in range(w):
                                        kt = g0 + i
                                        nc.tensor.matmul(
                                            oacc[:, 2 * idx + sub, :],
                                            lhsT=at[:, i * 256 + sub * 128:
                                                    i * 256 + sub * 128 + 128],
                                            rhs=vsb[:, kt, h * 65:(h + 1) * 65],
                                            start=(kt == lo[idx]),
                                            stop=(kt == kt_hi),
                                        )
                        for idx, h in enumerate(heads):
                            for sub in range(2):
                                oa = oacc[:, 2 * idx + sub, :]
                                rec = attp.tile([128, 1], FP32, tag="rec")
                                nc.vector.reciprocal(out=rec, in_=oa[:, 64:65])
                                rr = qt * 2 + sub
                                nc.vector.tensor_scalar_mul(
                                    out=o_rm[:, rr, h * 64:(h + 1) * 64],
                                    in0=oa[:, 0:64], scalar1=rec,
                                )
                    # O-projection for this q-tile's two row-tiles
                    for sub in range(2):
                        rt = 2 * qt + sub
                        ptile = ptro.tile([128, 4, 128], BF16, tag="pt",
                                          name=f"pt{rt}")
                        for c in range(4):
                            nc.tensor.transpose(
                                ptile[:, c, :], o_rm[:, rt, c * 128:(c + 1) * 128],
                                ident)
                        otT = op.tile([128, 4, 128], BF16, tag="otT",
                                      name=f"otT{rt}")
                        nc.vector.tensor_copy(out=otT, in_=ptile)
                        row_sb = op.tile([128, D], BF16, tag="row",
                                         name=f"row{rt}")
                        for n in range(2):
                            ps = pmm4.tile([128, 512], FP32, tag="ps",
                                           name=f"ps{rt}_{n}")
                            for c in range(4):
                                nc.tensor.matmul(
                                    ps,
                                    lhsT=otT[:, c, :],
                                    rhs=wo_sb[:, c, n * 512:(n + 1) * 512],
                                    start=(c == 0), stop=(c == 3),
                                )
                            nc.vector.tensor_copy(
                                out=row_sb[:, n * 512:(n + 1) * 512], in_=ps)
                        nc.sync.dma_start(
                            out=cc_in[cchalf][(gpos * 2 + sub) * 128:
                                              (gpos * 2 + sub) * 128 + 128, :],
                            in_=row_sb)
                nc.gpsimd.collective_compute(
                    "ReduceScatter",
                    mybir.AluOpType.add,
                    ins=[cc_in[cchalf].opt()],
                    outs=[cc_out[cchalf].opt()],
                    replica_groups=PAIRS,
                )

        p_big_cm.__exit__(None, None, None)
        p_x2_cm = tc.tile_pool(name="p_x2", bufs=1)          # phases 5-6
        p_x2 = p_x2_cm.__enter__()
        x2_sb = p_x2.tile([128, 8, D], BF16)
        h2g = [p_x2.tile([128, 8, 512], BF16, name=f"h2T{g}", tag=f"h2T{g}")
               for g in range(2)]

        # ---------------- Phases 5+6 interleaved per row-half ----------------
        with tc.tile_pool(name="ffnw", bufs=1) as ffnw, \
             tc.tile_pool(name="ln2", bufs=3) as ln2p, \
             tc.tile_pool(name="xmy", bufs=4) as xmyp, \
             tc.tile_pool(name="ptr5", bufs=4, space="PSUM") as ptr5, \
             tc.tile_pool(name="ffn2", bufs=3) as ffn2, \
             tc.tile_pool(name="pmm6", bufs=3, space="PSUM") as pmm6:
            w2_sb = ffnw.tile([128, 8, D], BF16, tag="w2")
            nc.sync.dma_start(out=w2_sb, in_=w2T.rearrange("(c p) n -> p c n", p=128))
            gs = ffnw.tile([128, 8, ROWS_OUT], BF16, tag="gs")
            for r2 in range(2):
                # phase 5 for this row half (gated on collective r2)
                for rt in range(4 * r2, 4 * r2 + 4):
                    xt = xmyp.tile([128, D], FP32, tag="xt", name=f"xt{rt}")
                    nc.sync.dma_start(out=xt, in_=x_my[rt * 128:(rt + 1) * 128, :])
                    rs = ln2p.tile([128, D], BF16, tag="rs")
                    nc.sync.dma_start(
                        out=rs,
                        in_=cc_out[r2][(rt % 4) * 128:(rt % 4) * 128 + 128, :])
                    nc.vector.tensor_add(x2_sb[:, rt, :], xt, rs)
                    stats = ln2p.tile([128, 2, 6], FP32, tag="stats")
                    x2r = x2_sb[:, rt, :].rearrange("p (s f) -> p s f", s=2)
                    nc.vector.bn_stats(out=stats[:, 0, :], in_=x2r[:, 0, :])
                    nc.vector.bn_stats(out=stats[:, 1, :], in_=x2r[:, 1, :])
                    mv = ln2p.tile([128, 2], FP32, tag="mv")
                    nc.vector.bn_aggr(out=mv, in_=stats)
                    rstd = ln2p.tile([128, 1], FP32, tag="rstd")
                    nc.scalar.activation(out=rstd, in_=mv[:, 1:2], func=AF.Sqrt,
                                         bias=eps)
                    nc.vector.reciprocal(out=rstd, in_=rstd)
                    hrow = ln2p.tile([128, D], BF16, tag="hrow")
                    nc.vector.tensor_scalar(
                        out=hrow, in0=x2_sb[:, rt, :], scalar1=mv[:, 0:1],
                        scalar2=rstd,
                        op0=mybir.AluOpType.subtract, op1=mybir.AluOpType.mult,
                    )
                    for c in range(8):
                        pt = ptr5.tile([128, 128], BF16, tag="pt")
                        nc.tensor.transpose(pt, hrow[:, c * 128:(c + 1) * 128], ident)
                        nc.vector.tensor_copy(
                            out=h2g[r2][:, c, (rt % 4) * 128:(rt % 4) * 128 + 128],
                            in_=pt)
                # FFN gate/up for this row half
                for f in range(8):
                    ps = pmm6.tile([128, 512], FP32, tag="ps")
                    for c in range(8):
                        nc.tensor.matmul(
                            ps,
                            lhsT=w1_sb[:, c, f * 128:(f + 1) * 128],
                            rhs=h2g[r2][:, c, :],
                            start=(c == 0), stop=(c == 7),
                        )
                    us = ffn2.tile([128, 512], BF16, tag="us")
                    nc.scalar.activation(
                        out=us, in_=ps, func=AF.Silu, bias=b1_sb[:, f:f + 1],
                    )
                    ps2 = pmm6.tile([128, 512], FP32, tag="ps")
                    for c in range(8):
                        nc.tensor.matmul(
                            ps2,
                            lhsT=w3_sb[:, c, f * 128:(f + 1) * 128],
                            rhs=h2g[r2][:, c, :],
                            start=(c == 0), stop=(c == 7),
                        )
                    ts = ffn2.tile([128, 512], BF16, tag="ts")
                    nc.vector.tensor_scalar(
                        out=ts, in0=ps2, scalar1=b3_sb[:, f:f + 1], scalar2=None,
                        op0=mybir.AluOpType.add,
                    )
                    nc.vector.tensor_mul(gs[:, f, r2 * 512:(r2 + 1) * 512], us, ts)
            # FFN down-projection + residual
            for rt in range(ROWS_OUT // 128):
                ysb = ffn2.tile([128, D], FP32, tag="ysb")
                for n in range(2):
                    ps = pmm6.tile([128, 512], FP32, tag="ps")
                    for f in range(8):
                        nc.tensor.matmul(
                            ps,
                            lhsT=gs[:, f, rt * 128:(rt + 1) * 128],
                            rhs=w2_sb[:, f, n * 512:(n + 1) * 512],
                            start=(f == 0), stop=(f == 7),
                        )
                    nc.vector.tensor_add(
                        ysb[:, n * 512:(n + 1) * 512], ps,
                        x2_sb[:, rt, n * 512:(n + 1) * 512],
                    )
                nc.sync.dma_start(out=y[rt * 128:(rt + 1) * 128, :], in_=ysb)

        p_x2_cm.__exit__(None, None, None)

    return nc

H = 16
_SLOPES = (2.0 ** (-8.0 / H)) ** np.arange(1, H + 1)
_RIDX = np.arange(S, dtype=np.float64)


def _prep_core_inputs(inp, core):
    b, half = core // 2, core % 2
    # slot i of this core holds head 2i+half (0-indexed)
    head_ids = [2 * i + half for i in range(NH)]
    dims = np.concatenate([np.arange(64 * h, 64 * h + 64) for h in head_ids])
    g1, b1 = f32(inp["ln1_g"]), f32(inp["ln1_b"])
    g2, b2 = f32(inp["ln2_g"]), f32(inp["ln2_b"])
    wq, wk, wv, wo = (f32(inp[k]) for k in ("wq", "wk", "wv", "wo"))
    w1, w2, w3 = (f32(inp[k]) for k in ("w1", "w2", "w3"))

    qkvbias = np.stack([
        (wq[dims] @ b1) / 8.0,
        wk[dims] @ b1,
        wv[dims] @ b1,
    ]).astype(np.float32)
    b13 = np.stack([w1 @ b2, w3 @ b2]).astype(np.float32)

    qext = np.zeros((NH, 4, S), BFNP)
    kext = np.zeros((NH, 4, S), BFNP)
    for j in range(NH):
        s_hat = float(bf(np.float32(_SLOPES[head_ids[j]])))
        qext[j, 0] = bf(np.full(S, s_hat))
        qext[j, 1] = bf(np.full(S, 256.0 * s_hat))
        qext[j, 2] = bf(-(_RIDX % 256))
        qext[j, 3] = bf(-(_RIDX // 256))
        kext[j, 0] = bf(_RIDX % 256)
        kext[j, 1] = bf(_RIDX // 256)
        kext[j, 2] = bf(np.full(S, s_hat))
        kext[j, 3] = bf(np.full(S, 256.0 * s_hat))

    out = {
        "x": f32(inp["x"][b]),
        "x_my": f32(inp["x"][b][half * ROWS_OUT:(half + 1) * ROWS_OUT]),
        "wqT": bf((wq[dims] * g1[None, :]).T),
        "wkT": bf((wk[dims] * g1[None, :]).T),
        "wvT": bf((wv[dims] * g1[None, :]).T),
        "woT": bf(wo[:, dims].T),
        "w1T": bf((w1 * g2[None, :]).T),
        "w3T": bf((w3 * g2[None, :]).T),
        "w2T": bf(w2.T),
        "qkvbias": qkvbias,
        "b13": b13,
        "qext": qext,
        "kext": kext,
    }
    return {k: np.ascontiguousarray(v) for k, v in out.items()}


_COMPILED_NC = None
LAST_EXEC_NS = None


def _get_nc():
    global _COMPILED_NC
    if _COMPILED_NC is None:
        nc = bacc.Bacc("TRN2", target_bir_lowering=False, debug=False,
                       num_devices=N_CORES)
        _build_kernel(nc)
        nc.compile()
        _COMPILED_NC = nc
    return _COMPILED_NC


def kernel(**inputs):
    global LAST_EXEC_NS
    nc = _get_nc()
    in_maps = [_prep_core_inputs(inputs, c) for c in range(N_CORES)]
    trace = os.environ.get("KERNEL_TRACE", "0") == "1"
    res = run_bass_kernel_spmd(nc, in_maps, list(range(N_CORES)), trace=trace)
    LAST_EXEC_NS = res.exec_time_ns
    B = inputs["x"].shape[0]
    y = np.zeros((B, S, D), np.float32)
    for core in range(N_CORES):
        b, half = core // 2, core % 2
        y[b, half * ROWS_OUT:(half + 1) * ROWS_OUT] = res.results[core]["y"]
    return y
